# revision 6
# baseline (speedup 1.0000x reference)
"""Trainium2 Bass kernel for Llama4TextExperts-style grouped MoE FFN (SwiGLU).

Full-input contract: kernel(**inputs) takes the complete unsharded tensors and
returns the full [4096, 1024] output. Internally: expert-parallel across the 8
NeuronCores — core e gets expert e's three weight matrices and that expert's
512-token group (tokens arrive pre-sorted by expert with equal group sizes).
All routing / transposition is done host-side in numpy; no collectives needed.

Per-core device program (three GEMMs, ~6.4 GFLOP):
  phase 1: gate^T = Wg^T-stationary @ x^T, up^T likewise; SwiGLU fused on
           ACT (Silu) + DVE (mul) into h^T [I, T] bf16 resident in SBUF.
  phase 2: y = h @ Wd with h^T slices stationary, Wd streaming from its
           natural [I, H] DRAM layout; y lands untransposed in PSUM.
"""

import numpy as np
import ml_dtypes

import concourse.bass as bass
import concourse.mybir as mybir
import concourse.tile as tile
from concourse import bacc
from concourse.bass_utils import run_bass_kernel_spmd

# Problem shape (hardcoded per contract)
E = 8          # experts == cores
T = 512        # tokens per expert group
H = 1024       # hidden
I = 2048       # intermediate
P = 128        # partitions
KT = H // P    # 8  k-tiles over hidden
IT = I // P    # 16 i-tiles over intermediate
WB = 4         # i-blocks of 512 columns for gate/up weight streaming
MT = T // P    # 4  token tiles

BF16 = mybir.dt.bfloat16
F32 = mybir.dt.float32

_compiled = None  # (nc, ) cached across calls


def _build():
    nc = bacc.Bacc(None)
    xT_d = nc.declare_dram_parameter("xT", [H, T], BF16, isOutput=False)
    wg_d = nc.declare_dram_parameter("wg", [H, I], BF16, isOutput=False)
    wu_d = nc.declare_dram_parameter("wu", [H, I], BF16, isOutput=False)
    wd_d = nc.declare_dram_parameter("wd", [I, H], BF16, isOutput=False)
    y_d = nc.declare_dram_parameter("y", [T, H], F32, isOutput=True)

    xT_r = xT_d.rearrange("(ko p) t -> p ko t", p=P)     # [128, 8, 512]
    wg_r = wg_d.rearrange("(ko p) i -> p ko i", p=P)     # [128, 8, 2048]
    wu_r = wu_d.rearrange("(ko p) i -> p ko i", p=P)
    wd_r = wd_d.rearrange("(io p) h -> p io h", p=P)     # [128, 16, 1024]

    with tile.TileContext(nc) as tc:
        with (
            tc.tile_pool(name="xpool", bufs=1) as xpool,
            tc.tile_pool(name="wdpool", bufs=1) as wdpool,
            tc.tile_pool(name="hpool", bufs=1) as hpool,
            tc.tile_pool(name="wpool", bufs=2) as wpool,
            tc.tile_pool(name="spool", bufs=3) as spool,
            tc.tile_pool(name="psum", bufs=2, space="PSUM") as psum,
        ):
            # x^T, chunked per k-tile so the first matmuls' deps clear as
            # soon as the first chunks land instead of after the full 1MB.
            xk = []
            for kt in range(KT):
                xc = xpool.tile([P, T], BF16, tag=f"x{kt}")
                nc.sync.dma_start(xc[:], xT_r[:, kt, :])
                xk.append(xc)

            hT_sb = hpool.tile([P, IT, T], BF16)
            wd_sb = wdpool.tile([P, IT, H], BF16)

            for wb in range(WB):
                # gate/up weight block, chunked per k-tile (separate tiles so
                # matmul k waits only on chunk k)
                wg_c, wu_c = [], []
                for kt in range(KT):
                    wgc = wpool.tile([P, 512], BF16, tag=f"wg{kt}")
                    nc.sync.dma_start(
                        wgc[:], wg_r[:, kt, wb * 512:(wb + 1) * 512]
                    )
                    wg_c.append(wgc)
                    wuc = wpool.tile([P, 512], BF16, tag=f"wu{kt}")
                    nc.sync.dma_start(
                        wuc[:], wu_r[:, kt, wb * 512:(wb + 1) * 512]
                    )
                    wu_c.append(wuc)
                # stream a quarter of Wd alongside each gate/up block, on the
                # gpsimd DMA queue so it doesn't contend with the critical path
                nc.gpsimd.dma_start(
                    wd_sb[:, wb * 4:(wb + 1) * 4, :],
                    wd_r[:, wb * 4:(wb + 1) * 4, :],
                )

                for itl in range(4):
                    it = wb * 4 + itl
                    cs = slice(itl * P, (itl + 1) * P)
                    pg = psum.tile([P, T], F32, tag="pg")
                    pu = psum.tile([P, T], F32, tag="pu")
                    for kt in range(KT):
                        nc.tensor.matmul(
                            pg[:], wg_c[kt][:, cs], xk[kt][:],
                            start=(kt == 0), stop=(kt == KT - 1),
                        )
                    for kt in range(KT):
                        nc.tensor.matmul(
                            pu[:], wu_c[kt][:, cs], xk[kt][:],
                            start=(kt == 0), stop=(kt == KT - 1),
                        )
                    sg = spool.tile([P, T], F32)
                    nc.scalar.activation(
                        sg[:], pg[:], mybir.ActivationFunctionType.Silu
                    )
                    nc.vector.tensor_mul(hT_sb[:, it, :], sg[:], pu[:])

            for mt in range(MT):
                py0 = psum.tile([P, 512], F32, tag="py0")
                py1 = psum.tile([P, 512], F32, tag="py1")
                ms = slice(mt * P, (mt + 1) * P)
                for it in range(IT):
                    lhsT = hT_sb[:, it, ms]
                    nc.tensor.matmul(
                        py0[:], lhsT, wd_sb[:, it, 0:512],
                        start=(it == 0), stop=(it == IT - 1),
                    )
                    nc.tensor.matmul(
                        py1[:], lhsT, wd_sb[:, it, 512:1024],
                        start=(it == 0), stop=(it == IT - 1),
                    )
                y0 = spool.tile([P, 512], F32, tag="y0")
                nc.scalar.copy(y0[:], py0[:])
                nc.sync.dma_start(y_d[ms, 0:512], y0[:])
                y1 = spool.tile([P, 512], F32, tag="y1")
                nc.vector.tensor_copy(y1[:], py1[:])
                nc.sync.dma_start(y_d[ms, 512:1024], y1[:])

    nc.compile()
    return nc


def _get_compiled():
    global _compiled
    if _compiled is None:
        _compiled = _build()
    return _compiled


def _numpy_fallback(hidden_states, gate_kernel, up_kernel, down_kernel, group_sizes):
    # Exact reference math on host; only used for unexpected group_sizes.
    out = np.empty((hidden_states.shape[0], down_kernel.shape[2]), np.float32)
    start = 0
    for e in range(gate_kernel.shape[0]):
        g = int(group_sizes[e])
        x = hidden_states[start:start + g]
        gate = x @ gate_kernel[e]
        up = x @ up_kernel[e]
        h = (gate / (1.0 + np.exp(-gate))) * up
        out[start:start + g] = h @ down_kernel[e]
        start += g
    out[start:] = 0.0
    return out


def _make_in_maps(hidden_states, gate_kernel, up_kernel, down_kernel):
    bf = ml_dtypes.bfloat16
    in_maps = []
    for e in range(E):
        x_e = hidden_states[e * T:(e + 1) * T]
        in_maps.append({
            "xT": np.ascontiguousarray(x_e.T).astype(bf),
            "wg": np.ascontiguousarray(gate_kernel[e]).astype(bf),
            "wu": np.ascontiguousarray(up_kernel[e]).astype(bf),
            "wd": np.ascontiguousarray(down_kernel[e]).astype(bf),
        })
    return in_maps


def profile_run(inputs, tmpdir=None):
    """Dev helper (not used by grading): run with NTFF tracing, return exec ns."""
    nc = _get_compiled()
    in_maps = _make_in_maps(
        np.asarray(inputs["hidden_states"], np.float32),
        np.asarray(inputs["gate_kernel"], np.float32),
        np.asarray(inputs["up_kernel"], np.float32),
        np.asarray(inputs["down_kernel"], np.float32),
    )
    res = run_bass_kernel_spmd(
        nc, in_maps, core_ids=list(range(E)), trace=True, tmpdir=tmpdir
    )
    return res.exec_time_ns


def kernel(hidden_states, gate_kernel, up_kernel, down_kernel, group_sizes):
    hidden_states = np.asarray(hidden_states, dtype=np.float32)
    gate_kernel = np.asarray(gate_kernel, dtype=np.float32)
    up_kernel = np.asarray(up_kernel, dtype=np.float32)
    down_kernel = np.asarray(down_kernel, dtype=np.float32)
    gs = np.asarray(group_sizes)

    if not (gs.shape == (E,) and np.all(gs == T)):
        return _numpy_fallback(
            hidden_states, gate_kernel, up_kernel, down_kernel, gs
        )

    nc = _get_compiled()
    in_maps = _make_in_maps(hidden_states, gate_kernel, up_kernel, down_kernel)
    res = run_bass_kernel_spmd(nc, in_maps, core_ids=list(range(E)))
    return np.concatenate([res.results[e]["y"] for e in range(E)], axis=0)


# revision 7
# speedup vs baseline: 1.1434x; 1.1434x over previous
"""Trainium2 Bass kernel for Llama4TextExperts-style grouped MoE FFN (SwiGLU).

Full-input contract: kernel(**inputs) takes the complete unsharded tensors and
returns the full [4096, 1024] output. Internally: expert-parallel across the 8
NeuronCores — core e gets expert e's three weight matrices and that expert's
512-token group (tokens arrive pre-sorted by expert with equal group sizes).
All routing / transposition is done host-side in numpy; no collectives needed.

Per-core device program (three GEMMs, ~6.4 GFLOP):
  phase 1: gate^T = Wg^T-stationary @ x^T, up^T likewise; SwiGLU fused on
           ACT (Silu) + DVE (mul) into h^T [I, T] bf16 resident in SBUF.
  phase 2: y = h @ Wd with h^T slices stationary, Wd streaming from its
           natural [I, H] DRAM layout; y lands untransposed in PSUM.
"""

import numpy as np
import ml_dtypes

import concourse.bass as bass
import concourse.mybir as mybir
import concourse.tile as tile
from concourse import bacc
from concourse.bass_utils import run_bass_kernel_spmd

# Problem shape (hardcoded per contract)
E = 8          # experts == cores
T = 512        # tokens per expert group
H = 1024       # hidden
I = 2048       # intermediate
P = 128        # partitions
KT = H // P    # 8  k-tiles over hidden
IT = I // P    # 16 i-tiles over intermediate
WB = 4         # i-blocks of 512 columns for gate/up weight streaming
MT = T // P    # 4  token tiles

BF16 = mybir.dt.bfloat16
F32 = mybir.dt.float32

_compiled = None  # (nc, ) cached across calls


def _build():
    nc = bacc.Bacc(None)
    xT_d = nc.declare_dram_parameter("xT", [H, T], BF16, isOutput=False)
    wg_d = nc.declare_dram_parameter("wg", [H, I], BF16, isOutput=False)
    wu_d = nc.declare_dram_parameter("wu", [H, I], BF16, isOutput=False)
    wd_d = nc.declare_dram_parameter("wd", [I, H], BF16, isOutput=False)
    y_d = nc.declare_dram_parameter("y", [T, H], F32, isOutput=True)

    xT_r = xT_d.rearrange("(ko p) t -> p ko t", p=P)     # [128, 8, 512]
    wg_r = wg_d.rearrange("(ko p) i -> p ko i", p=P)     # [128, 8, 2048]
    wu_r = wu_d.rearrange("(ko p) i -> p ko i", p=P)
    wd_r = wd_d.rearrange("(io p) h -> p io h", p=P)     # [128, 16, 1024]

    with tile.TileContext(nc) as tc:
        with (
            tc.tile_pool(name="xpool", bufs=1) as xpool,
            tc.tile_pool(name="wdpool", bufs=1) as wdpool,
            tc.tile_pool(name="hpool", bufs=1) as hpool,
            tc.tile_pool(name="wpool", bufs=3) as wpool,
            tc.tile_pool(name="spool", bufs=3) as spool,
            tc.tile_pool(name="psum", bufs=2, space="PSUM") as psum,
        ):
            # Startup-critical loads, ordered so the first matmuls' deps land
            # first. The gate/up weights for the very first i-tile go in two
            # narrow descriptors at the head of the sync HWDGE queue; x^T is
            # chunked per k-tile on the scalar HWDGE queue so both queues
            # issue in parallel.
            wg_b0_it0 = xpool.tile([P, KT, 128], BF16, tag="wg_b0_it0")
            nc.sync.dma_start(wg_b0_it0[:], wg_r[:, :, 0:128])
            wu_b0_it0 = xpool.tile([P, KT, 128], BF16, tag="wu_b0_it0")
            nc.sync.dma_start(wu_b0_it0[:], wu_r[:, :, 0:128])
            wg_b0_rest = xpool.tile([P, KT, 384], BF16, tag="wg_b0_rest")
            nc.sync.dma_start(wg_b0_rest[:], wg_r[:, :, 128:512])
            wu_b0_rest = xpool.tile([P, KT, 384], BF16, tag="wu_b0_rest")
            nc.sync.dma_start(wu_b0_rest[:], wu_r[:, :, 128:512])

            xk = []
            for kt in range(KT):
                xc = xpool.tile([P, T], BF16, tag=f"x{kt}")
                nc.scalar.dma_start(xc[:], xT_r[:, kt, :])
                xk.append(xc)

            hT_sb = hpool.tile([P, IT, T], BF16)
            wd_sb = wdpool.tile([P, IT, H], BF16)

            # gate/up blocks 1..3 (full 512-wide), then Wd at the back of the
            # sync queue — needed only by phase 2.
            wgf, wuf = {}, {}
            for wb in range(1, WB):
                wgb = wpool.tile([P, KT, 512], BF16, tag="wgf")
                nc.sync.dma_start(wgb[:], wg_r[:, :, wb * 512:(wb + 1) * 512])
                wgf[wb] = wgb
                wub = wpool.tile([P, KT, 512], BF16, tag="wuf")
                nc.sync.dma_start(wub[:], wu_r[:, :, wb * 512:(wb + 1) * 512])
                wuf[wb] = wub
            for wb in range(WB):
                nc.sync.dma_start(
                    wd_sb[:, wb * 4:(wb + 1) * 4, :],
                    wd_r[:, wb * 4:(wb + 1) * 4, :],
                )

            def gu_slice(wb, itl, which):
                if wb == 0:
                    if itl == 0:
                        t = wg_b0_it0 if which == "g" else wu_b0_it0
                        return lambda kt: t[:, kt, :]
                    t = wg_b0_rest if which == "g" else wu_b0_rest
                    return lambda kt: t[:, kt, (itl - 1) * P:itl * P]
                t = wgf[wb] if which == "g" else wuf[wb]
                return lambda kt: t[:, kt, itl * P:(itl + 1) * P]

            for wb in range(WB):
                for itl in range(4):
                    it = wb * 4 + itl
                    gsl = gu_slice(wb, itl, "g")
                    usl = gu_slice(wb, itl, "u")
                    pg = psum.tile([P, T], F32, tag="pg")
                    pu = psum.tile([P, T], F32, tag="pu")
                    for kt in range(KT):
                        nc.tensor.matmul(
                            pg[:], gsl(kt), xk[kt][:],
                            start=(kt == 0), stop=(kt == KT - 1),
                        )
                    for kt in range(KT):
                        nc.tensor.matmul(
                            pu[:], usl(kt), xk[kt][:],
                            start=(kt == 0), stop=(kt == KT - 1),
                        )
                    sg = spool.tile([P, T], F32)
                    nc.scalar.activation(
                        sg[:], pg[:], mybir.ActivationFunctionType.Silu
                    )
                    nc.vector.tensor_mul(hT_sb[:, it, :], sg[:], pu[:])

            for mt in range(MT):
                py0 = psum.tile([P, 512], F32, tag="py0")
                py1 = psum.tile([P, 512], F32, tag="py1")
                ms = slice(mt * P, (mt + 1) * P)
                for it in range(IT):
                    lhsT = hT_sb[:, it, ms]
                    nc.tensor.matmul(
                        py0[:], lhsT, wd_sb[:, it, 0:512],
                        start=(it == 0), stop=(it == IT - 1),
                    )
                    nc.tensor.matmul(
                        py1[:], lhsT, wd_sb[:, it, 512:1024],
                        start=(it == 0), stop=(it == IT - 1),
                    )
                y0 = spool.tile([P, 512], F32, tag="y0")
                nc.scalar.copy(y0[:], py0[:])
                nc.sync.dma_start(y_d[ms, 0:512], y0[:])
                y1 = spool.tile([P, 512], F32, tag="y1")
                nc.vector.tensor_copy(y1[:], py1[:])
                nc.sync.dma_start(y_d[ms, 512:1024], y1[:])

    nc.compile()
    return nc


def _get_compiled():
    global _compiled
    if _compiled is None:
        _compiled = _build()
    return _compiled


def _numpy_fallback(hidden_states, gate_kernel, up_kernel, down_kernel, group_sizes):
    # Exact reference math on host; only used for unexpected group_sizes.
    out = np.empty((hidden_states.shape[0], down_kernel.shape[2]), np.float32)
    start = 0
    for e in range(gate_kernel.shape[0]):
        g = int(group_sizes[e])
        x = hidden_states[start:start + g]
        gate = x @ gate_kernel[e]
        up = x @ up_kernel[e]
        h = (gate / (1.0 + np.exp(-gate))) * up
        out[start:start + g] = h @ down_kernel[e]
        start += g
    out[start:] = 0.0
    return out


def _make_in_maps(hidden_states, gate_kernel, up_kernel, down_kernel):
    bf = ml_dtypes.bfloat16
    in_maps = []
    for e in range(E):
        x_e = hidden_states[e * T:(e + 1) * T]
        in_maps.append({
            "xT": np.ascontiguousarray(x_e.T).astype(bf),
            "wg": np.ascontiguousarray(gate_kernel[e]).astype(bf),
            "wu": np.ascontiguousarray(up_kernel[e]).astype(bf),
            "wd": np.ascontiguousarray(down_kernel[e]).astype(bf),
        })
    return in_maps


def profile_run(inputs, tmpdir=None):
    """Dev helper (not used by grading): run with NTFF tracing, return exec ns."""
    nc = _get_compiled()
    in_maps = _make_in_maps(
        np.asarray(inputs["hidden_states"], np.float32),
        np.asarray(inputs["gate_kernel"], np.float32),
        np.asarray(inputs["up_kernel"], np.float32),
        np.asarray(inputs["down_kernel"], np.float32),
    )
    res = run_bass_kernel_spmd(
        nc, in_maps, core_ids=list(range(E)), trace=True, tmpdir=tmpdir
    )
    return res.exec_time_ns


def kernel(hidden_states, gate_kernel, up_kernel, down_kernel, group_sizes):
    hidden_states = np.asarray(hidden_states, dtype=np.float32)
    gate_kernel = np.asarray(gate_kernel, dtype=np.float32)
    up_kernel = np.asarray(up_kernel, dtype=np.float32)
    down_kernel = np.asarray(down_kernel, dtype=np.float32)
    gs = np.asarray(group_sizes)

    if not (gs.shape == (E,) and np.all(gs == T)):
        return _numpy_fallback(
            hidden_states, gate_kernel, up_kernel, down_kernel, gs
        )

    nc = _get_compiled()
    in_maps = _make_in_maps(hidden_states, gate_kernel, up_kernel, down_kernel)
    res = run_bass_kernel_spmd(nc, in_maps, core_ids=list(range(E)))
    return np.concatenate([res.results[e]["y"] for e in range(E)], axis=0)


# revision 10
# speedup vs baseline: 1.1464x; 1.0026x over previous
"""Trainium2 Bass kernel for Llama4TextExperts-style grouped MoE FFN (SwiGLU).

Full-input contract: kernel(**inputs) takes the complete unsharded tensors and
returns the full [4096, 1024] output. Internally: expert-parallel across the 8
NeuronCores — core e gets expert e's three weight matrices and that expert's
512-token group (tokens arrive pre-sorted by expert with equal group sizes).
All routing / transposition is done host-side in numpy; no collectives needed.

Per-core device program (three GEMMs, ~6.4 GFLOP):
  phase 1: gate^T = Wg^T-stationary @ x^T, up^T likewise; SwiGLU fused on
           ACT (Silu) + DVE (mul) into h^T [I, T] bf16 resident in SBUF.
  phase 2: y = h @ Wd with h^T slices stationary, Wd streaming from its
           natural [I, H] DRAM layout; y lands untransposed in PSUM.
"""

import numpy as np
import ml_dtypes

import concourse.bass as bass
import concourse.mybir as mybir
import concourse.tile as tile
from concourse.tile import add_dep_helper
from concourse import bacc
from concourse.bass_utils import run_bass_kernel_spmd

# Problem shape (hardcoded per contract)
E = 8          # experts == cores
T = 512        # tokens per expert group
H = 1024       # hidden
I = 2048       # intermediate
P = 128        # partitions
KT = H // P    # 8  k-tiles over hidden
IT = I // P    # 16 i-tiles over intermediate
WB = 4         # i-blocks of 512 columns for gate/up weight streaming
MT = T // P    # 4  token tiles

BF16 = mybir.dt.bfloat16
F32 = mybir.dt.float32

_compiled = None  # (nc, ) cached across calls


def _build():
    nc = bacc.Bacc(None)
    xT_d = nc.declare_dram_parameter("xT", [H, T], BF16, isOutput=False)
    wg_d = nc.declare_dram_parameter("wg", [H, I], BF16, isOutput=False)
    wu_d = nc.declare_dram_parameter("wu", [H, I], BF16, isOutput=False)
    wd_d = nc.declare_dram_parameter("wd", [I, H], BF16, isOutput=False)
    y_d = nc.declare_dram_parameter("y", [T, H], F32, isOutput=True)

    xT_r = xT_d.rearrange("(ko p) t -> p ko t", p=P)     # [128, 8, 512]
    wg_r = wg_d.rearrange("(ko p) i -> p ko i", p=P)     # [128, 8, 2048]
    wu_r = wu_d.rearrange("(ko p) i -> p ko i", p=P)
    wd_r = wd_d.rearrange("(io p) h -> p io h", p=P)     # [128, 16, 1024]

    with tile.TileContext(nc) as tc:
        with (
            tc.tile_pool(name="xpool", bufs=1) as xpool,
            tc.tile_pool(name="wdpool", bufs=1) as wdpool,
            tc.tile_pool(name="hpool", bufs=1) as hpool,
            tc.tile_pool(name="wpool", bufs=3) as wpool,
            tc.tile_pool(name="spool", bufs=3) as spool,
            tc.tile_pool(name="psum", bufs=2, space="PSUM") as psum,
        ):
            # Startup-critical loads. The DMA engines round-robin across all
            # outstanding descriptors (everything in flight completes
            # together at ~320GB/s aggregate), so later weight loads are
            # GATED on earlier DMA completions: the pipe first carries only
            # the ~1.5MB the first i-tile needs, then stays about one
            # compute-block ahead.
            wg_it = []   # per-i-tile gate weights for block 0
            wu_it = []
            d_stage = []  # last DMA instruction of each stage, for gating
            wg_it.append(xpool.tile([P, KT, 128], BF16, tag="wg_it0", name="wg_it0"))
            d0g = nc.sync.dma_start(wg_it[0][:], wg_r[:, :, 0:128])
            wu_it.append(xpool.tile([P, KT, 128], BF16, tag="wu_it0", name="wu_it0"))
            d0u = nc.sync.dma_start(wu_it[0][:], wu_r[:, :, 0:128])

            xk4 = []
            for kq in range(4):
                xc = xpool.tile([P, 2, T], BF16, tag=f"x{kq}", name=f"x{kq}")
                nc.scalar.dma_start(xc[:], xT_r[:, 2 * kq:2 * kq + 2, :])
                xk4.append(xc)

            def xk(kt):
                return xk4[kt // 2][:, kt % 2, :]

            hT_sb = hpool.tile([P, IT, T], BF16)
            wd_sb = wdpool.tile([P, IT, H], BF16)

            def gated(dma_call, stage_idx):
                if stage_idx >= 0:
                    add_dep_helper(
                        dma_call.ins, d_stage[stage_idx].ins,
                        reason="dma staging throttle",
                    )
                return dma_call

            # stage 0: it1 of block 0
            wg_it.append(xpool.tile([P, KT, 128], BF16, tag="wg_it1", name="wg_it1"))
            gated(nc.sync.dma_start(wg_it[1][:], wg_r[:, :, 128:256]), -1)
            wu_it.append(xpool.tile([P, KT, 128], BF16, tag="wu_it1", name="wu_it1"))
            d_stage.append(
                gated(nc.sync.dma_start(wu_it[1][:], wu_r[:, :, 128:256]), -1)
            )
            # stage 1: it2+it3 of block 0, gated on stage 0
            wg_b0r = xpool.tile([P, KT, 256], BF16, tag="wg_b0r")
            gated(nc.sync.dma_start(wg_b0r[:], wg_r[:, :, 256:512]), 0)
            wu_b0r = xpool.tile([P, KT, 256], BF16, tag="wu_b0r")
            d_stage.append(
                gated(nc.sync.dma_start(wu_b0r[:], wu_r[:, :, 256:512]), 0)
            )
            # stages 2..4: blocks 1..3, each gated on the previous stage
            wgf, wuf = {}, {}
            for wb in range(1, WB):
                wgb = wpool.tile([P, KT, 512], BF16, tag="wgf")
                gated(
                    nc.sync.dma_start(
                        wgb[:], wg_r[:, :, wb * 512:(wb + 1) * 512]
                    ),
                    wb - 1,
                )
                wgf[wb] = wgb
                wub = wpool.tile([P, KT, 512], BF16, tag="wuf")
                d_stage.append(
                    gated(
                        nc.sync.dma_start(
                            wub[:], wu_r[:, :, wb * 512:(wb + 1) * 512]
                        ),
                        wb - 1,
                    )
                )
                wuf[wb] = wub
            # stages 5,6: Wd halves, chained behind the weight stream
            d_stage.append(
                gated(
                    nc.sync.dma_start(wd_sb[:, 0:8, :], wd_r[:, 0:8, :]), 3
                )
            )
            gated(nc.sync.dma_start(wd_sb[:, 8:16, :], wd_r[:, 8:16, :]), 4)

            def gu_slice(wb, itl, which):
                if wb == 0:
                    if itl < 2:
                        t = wg_it[itl] if which == "g" else wu_it[itl]
                        return lambda kt: t[:, kt, :]
                    t = wg_b0r if which == "g" else wu_b0r
                    return lambda kt: t[:, kt, (itl - 2) * P:(itl - 1) * P]
                t = wgf[wb] if which == "g" else wuf[wb]
                return lambda kt: t[:, kt, itl * P:(itl + 1) * P]

            for wb in range(WB):
                for itl in range(4):
                    it = wb * 4 + itl
                    gsl = gu_slice(wb, itl, "g")
                    usl = gu_slice(wb, itl, "u")
                    pg = psum.tile([P, T], F32, tag="pg")
                    pu = psum.tile([P, T], F32, tag="pu")
                    for kt in range(KT):
                        nc.tensor.matmul(
                            pg[:], gsl(kt), xk(kt),
                            start=(kt == 0), stop=(kt == KT - 1),
                        )
                    for kt in range(KT):
                        nc.tensor.matmul(
                            pu[:], usl(kt), xk(kt),
                            start=(kt == 0), stop=(kt == KT - 1),
                        )
                    sg = spool.tile([P, T], F32)
                    nc.scalar.activation(
                        sg[:], pg[:], mybir.ActivationFunctionType.Silu
                    )
                    nc.vector.tensor_mul(hT_sb[:, it, :], sg[:], pu[:])

            for mt in range(MT):
                ms = slice(mt * P, (mt + 1) * P)
                if mt < MT - 1:
                    py0 = psum.tile([P, 512], F32, tag="py0")
                    py1 = psum.tile([P, 512], F32, tag="py1")
                    for it in range(IT):
                        lhsT = hT_sb[:, it, ms]
                        nc.tensor.matmul(
                            py0[:], lhsT, wd_sb[:, it, 0:512],
                            start=(it == 0), stop=(it == IT - 1),
                        )
                        nc.tensor.matmul(
                            py1[:], lhsT, wd_sb[:, it, 512:1024],
                            start=(it == 0), stop=(it == IT - 1),
                        )
                    y0 = spool.tile([P, 512], F32, tag="y0")
                    nc.scalar.copy(y0[:], py0[:])
                    nc.sync.dma_start(y_d[ms, 0:512], y0[:])
                    y1 = spool.tile([P, 512], F32, tag="y1")
                    nc.vector.tensor_copy(y1[:], py1[:])
                    nc.sync.dma_start(y_d[ms, 512:1024], y1[:])
                else:
                    # last token tile: run the two 16-matmul chains
                    # back-to-back instead of interleaved, so the first
                    # half's copy+DMA overlaps the second half's matmuls and
                    # only one [128,512] copy+DMA remains after the last MM.
                    py0 = psum.tile([P, 512], F32, tag="py0")
                    for it in range(IT):
                        nc.tensor.matmul(
                            py0[:], hT_sb[:, it, ms], wd_sb[:, it, 0:512],
                            start=(it == 0), stop=(it == IT - 1),
                        )
                    y0 = spool.tile([P, 512], F32, tag="y0")
                    nc.scalar.copy(y0[:], py0[:])
                    nc.sync.dma_start(y_d[ms, 0:512], y0[:])
                    py1 = psum.tile([P, 512], F32, tag="py1")
                    for it in range(IT):
                        nc.tensor.matmul(
                            py1[:], hT_sb[:, it, ms], wd_sb[:, it, 512:1024],
                            start=(it == 0), stop=(it == IT - 1),
                        )
                    y1 = spool.tile([P, 512], F32, tag="y1")
                    nc.vector.tensor_copy(y1[:], py1[:])
                    nc.sync.dma_start(y_d[ms, 512:1024], y1[:])

    nc.compile()
    return nc


def _get_compiled():
    global _compiled
    if _compiled is None:
        _compiled = _build()
    return _compiled


def _numpy_fallback(hidden_states, gate_kernel, up_kernel, down_kernel, group_sizes):
    # Exact reference math on host; only used for unexpected group_sizes.
    out = np.empty((hidden_states.shape[0], down_kernel.shape[2]), np.float32)
    start = 0
    for e in range(gate_kernel.shape[0]):
        g = int(group_sizes[e])
        x = hidden_states[start:start + g]
        gate = x @ gate_kernel[e]
        up = x @ up_kernel[e]
        h = (gate / (1.0 + np.exp(-gate))) * up
        out[start:start + g] = h @ down_kernel[e]
        start += g
    out[start:] = 0.0
    return out


def _make_in_maps(hidden_states, gate_kernel, up_kernel, down_kernel):
    bf = ml_dtypes.bfloat16
    in_maps = []
    for e in range(E):
        x_e = hidden_states[e * T:(e + 1) * T]
        in_maps.append({
            "xT": np.ascontiguousarray(x_e.T).astype(bf),
            "wg": np.ascontiguousarray(gate_kernel[e]).astype(bf),
            "wu": np.ascontiguousarray(up_kernel[e]).astype(bf),
            "wd": np.ascontiguousarray(down_kernel[e]).astype(bf),
        })
    return in_maps


def profile_run(inputs, tmpdir=None):
    """Dev helper (not used by grading): run with NTFF tracing, return exec ns."""
    nc = _get_compiled()
    in_maps = _make_in_maps(
        np.asarray(inputs["hidden_states"], np.float32),
        np.asarray(inputs["gate_kernel"], np.float32),
        np.asarray(inputs["up_kernel"], np.float32),
        np.asarray(inputs["down_kernel"], np.float32),
    )
    res = run_bass_kernel_spmd(
        nc, in_maps, core_ids=list(range(E)), trace=True, tmpdir=tmpdir
    )
    return res.exec_time_ns


def kernel(hidden_states, gate_kernel, up_kernel, down_kernel, group_sizes):
    hidden_states = np.asarray(hidden_states, dtype=np.float32)
    gate_kernel = np.asarray(gate_kernel, dtype=np.float32)
    up_kernel = np.asarray(up_kernel, dtype=np.float32)
    down_kernel = np.asarray(down_kernel, dtype=np.float32)
    gs = np.asarray(group_sizes)

    if not (gs.shape == (E,) and np.all(gs == T)):
        return _numpy_fallback(
            hidden_states, gate_kernel, up_kernel, down_kernel, gs
        )

    nc = _get_compiled()
    in_maps = _make_in_maps(hidden_states, gate_kernel, up_kernel, down_kernel)
    res = run_bass_kernel_spmd(nc, in_maps, core_ids=list(range(E)))
    return np.concatenate([res.results[e]["y"] for e in range(E)], axis=0)


# revision 13
# speedup vs baseline: 1.1486x; 1.0019x over previous
"""Trainium2 Bass kernel for Llama4TextExperts-style grouped MoE FFN (SwiGLU).

Full-input contract: kernel(**inputs) takes the complete unsharded tensors and
returns the full [4096, 1024] output. Internally: expert-parallel across the 8
NeuronCores — core e gets expert e's three weight matrices and that expert's
512-token group (tokens arrive pre-sorted by expert with equal group sizes).
All routing / transposition is done host-side in numpy; no collectives needed.

Per-core device program (three GEMMs, ~6.4 GFLOP):
  phase 1: gate^T = Wg^T-stationary @ x^T, up^T likewise; SwiGLU fused on
           ACT (Silu) + DVE (mul) into h^T [I, T] bf16 resident in SBUF.
  phase 2: y = h @ Wd with h^T slices stationary, Wd streaming from its
           natural [I, H] DRAM layout; y lands untransposed in PSUM.
"""

import numpy as np
import ml_dtypes

import concourse.bass as bass
import concourse.mybir as mybir
import concourse.tile as tile
from concourse.tile import add_dep_helper
from concourse import bacc
from concourse.bass_utils import run_bass_kernel_spmd

# Problem shape (hardcoded per contract)
E = 8          # experts == cores
T = 512        # tokens per expert group
H = 1024       # hidden
I = 2048       # intermediate
P = 128        # partitions
KT = H // P    # 8  k-tiles over hidden
IT = I // P    # 16 i-tiles over intermediate
WB = 4         # i-blocks of 512 columns for gate/up weight streaming
MT = T // P    # 4  token tiles

BF16 = mybir.dt.bfloat16
F32 = mybir.dt.float32

_compiled = None  # (nc, ) cached across calls


def _build():
    nc = bacc.Bacc(None)
    xT_d = nc.declare_dram_parameter("xT", [H, T], BF16, isOutput=False)
    wg_d = nc.declare_dram_parameter("wg", [H, I], BF16, isOutput=False)
    wu_d = nc.declare_dram_parameter("wu", [H, I], BF16, isOutput=False)
    wd_d = nc.declare_dram_parameter("wd", [I, H], BF16, isOutput=False)
    y_d = nc.declare_dram_parameter("y", [T, H], F32, isOutput=True)

    xT_r = xT_d.rearrange("(ko p) t -> p ko t", p=P)     # [128, 8, 512]
    wg_r = wg_d.rearrange("(ko p) i -> p ko i", p=P)     # [128, 8, 2048]
    wu_r = wu_d.rearrange("(ko p) i -> p ko i", p=P)
    wd_r = wd_d.rearrange("(io p) h -> p io h", p=P)     # [128, 16, 1024]

    with tile.TileContext(nc) as tc:
        with (
            tc.tile_pool(name="xpool", bufs=1) as xpool,
            tc.tile_pool(name="wdpool", bufs=1) as wdpool,
            tc.tile_pool(name="hpool", bufs=1) as hpool,
            tc.tile_pool(name="wpool", bufs=3) as wpool,
            tc.tile_pool(name="spool", bufs=3) as spool,
            tc.tile_pool(name="psum", bufs=2, space="PSUM") as psum,
        ):
            # Startup-critical loads. The DMA engines round-robin across all
            # outstanding descriptors (everything in flight completes
            # together at ~320GB/s aggregate), so later weight loads are
            # GATED on earlier DMA completions: the pipe first carries only
            # the ~1.5MB the first i-tile needs, then stays about one
            # compute-block ahead.
            wg_it = []   # per-i-tile gate weights for block 0
            wu_it = []
            d_stage = []  # last DMA instruction of each stage, for gating
            wg_it.append(xpool.tile([P, KT, 128], BF16, tag="wg_it0", name="wg_it0"))
            d0g = nc.sync.dma_start(wg_it[0][:], wg_r[:, :, 0:128])
            wu_it.append(xpool.tile([P, KT, 128], BF16, tag="wu_it0", name="wu_it0"))
            d0u = nc.sync.dma_start(wu_it[0][:], wu_r[:, :, 0:128])

            xk4 = []
            for kq in range(4):
                xc = xpool.tile([P, 2, T], BF16, tag=f"x{kq}", name=f"x{kq}")
                nc.scalar.dma_start(xc[:], xT_r[:, 2 * kq:2 * kq + 2, :])
                xk4.append(xc)

            def xk(kt):
                return xk4[kt // 2][:, kt % 2, :]

            hT_sb = hpool.tile([P, IT, T], BF16)
            wd_sb = wdpool.tile([P, IT, H], BF16)

            def gated(dma_call, stage_idx, dep=None):
                if dep is not None:
                    add_dep_helper(
                        dma_call.ins, dep.ins, reason="dma staging throttle"
                    )
                elif stage_idx >= 0:
                    add_dep_helper(
                        dma_call.ins, d_stage[stage_idx].ins,
                        reason="dma staging throttle",
                    )
                return dma_call

            # stage A (ungated, with xT): it1 of block 0
            wg_it.append(xpool.tile([P, KT, 128], BF16, tag="wg_it1", name="wg_it1"))
            gated(nc.sync.dma_start(wg_it[1][:], wg_r[:, :, 128:256]), -1)
            wu_it.append(xpool.tile([P, KT, 128], BF16, tag="wu_it1", name="wu_it1"))
            d_stage.append(
                gated(nc.sync.dma_start(wu_it[1][:], wu_r[:, :, 128:256]), -1)
            )
            # it2+it3 of block 0, gated on the first it0 load (lookahead
            # gating keeps the pipe from draining dry between stages)
            wg_b0r = xpool.tile([P, KT, 256], BF16, tag="wg_b0r")
            gated(nc.sync.dma_start(wg_b0r[:], wg_r[:, :, 256:512]), -2, d0g)
            wu_b0r = xpool.tile([P, KT, 256], BF16, tag="wu_b0r")
            d_stage.append(
                gated(nc.sync.dma_start(wu_b0r[:], wu_r[:, :, 256:512]), -2, d0g)
            )
            # blocks 1..3: block wb gated on the stage two back
            wgf, wuf = {}, {}
            for wb in range(1, WB):
                wgb = wpool.tile([P, KT, 512], BF16, tag="wgf")
                gated(
                    nc.sync.dma_start(
                        wgb[:], wg_r[:, :, wb * 512:(wb + 1) * 512]
                    ),
                    wb - 1,
                )
                wgf[wb] = wgb
                wub = wpool.tile([P, KT, 512], BF16, tag="wuf")
                d_stage.append(
                    gated(
                        nc.sync.dma_start(
                            wub[:], wu_r[:, :, wb * 512:(wb + 1) * 512]
                        ),
                        wb - 1,
                    )
                )
                wuf[wb] = wub
            # Wd halves, chained behind the weight stream
            d_stage.append(
                gated(
                    nc.sync.dma_start(wd_sb[:, 0:8, :], wd_r[:, 0:8, :]), 3
                )
            )
            gated(nc.sync.dma_start(wd_sb[:, 8:16, :], wd_r[:, 8:16, :]), 4)

            def gu_slice(wb, itl, which):
                if wb == 0:
                    if itl < 2:
                        t = wg_it[itl] if which == "g" else wu_it[itl]
                        return lambda kt: t[:, kt, :]
                    t = wg_b0r if which == "g" else wu_b0r
                    return lambda kt: t[:, kt, (itl - 2) * P:(itl - 1) * P]
                t = wgf[wb] if which == "g" else wuf[wb]
                return lambda kt: t[:, kt, itl * P:(itl + 1) * P]

            for wb in range(WB):
                for itl in range(4):
                    it = wb * 4 + itl
                    gsl = gu_slice(wb, itl, "g")
                    usl = gu_slice(wb, itl, "u")
                    pg = psum.tile([P, T], F32, tag="pg")
                    pu = psum.tile([P, T], F32, tag="pu")
                    for kt in range(KT):
                        nc.tensor.matmul(
                            pg[:], gsl(kt), xk(kt),
                            start=(kt == 0), stop=(kt == KT - 1),
                        )
                    for kt in range(KT):
                        nc.tensor.matmul(
                            pu[:], usl(kt), xk(kt),
                            start=(kt == 0), stop=(kt == KT - 1),
                        )
                    sg = spool.tile([P, T], F32)
                    nc.scalar.activation(
                        sg[:], pg[:], mybir.ActivationFunctionType.Silu
                    )
                    nc.vector.tensor_mul(hT_sb[:, it, :], sg[:], pu[:])

            for mt in range(MT):
                ms = slice(mt * P, (mt + 1) * P)
                if mt < MT - 1:
                    py0 = psum.tile([P, 512], F32, tag="py0")
                    py1 = psum.tile([P, 512], F32, tag="py1")
                    for it in range(IT):
                        lhsT = hT_sb[:, it, ms]
                        nc.tensor.matmul(
                            py0[:], lhsT, wd_sb[:, it, 0:512],
                            start=(it == 0), stop=(it == IT - 1),
                        )
                        nc.tensor.matmul(
                            py1[:], lhsT, wd_sb[:, it, 512:1024],
                            start=(it == 0), stop=(it == IT - 1),
                        )
                    y0 = spool.tile([P, 512], F32, tag="y0")
                    nc.scalar.copy(y0[:], py0[:])
                    nc.sync.dma_start(y_d[ms, 0:512], y0[:])
                    y1 = spool.tile([P, 512], F32, tag="y1")
                    nc.vector.tensor_copy(y1[:], py1[:])
                    nc.sync.dma_start(y_d[ms, 512:1024], y1[:])
                else:
                    # last token tile: run the two 16-matmul chains
                    # back-to-back instead of interleaved, so the first
                    # half's copy+DMA overlaps the second half's matmuls and
                    # only one [128,512] copy+DMA remains after the last MM.
                    py0 = psum.tile([P, 512], F32, tag="py0")
                    for it in range(IT):
                        nc.tensor.matmul(
                            py0[:], hT_sb[:, it, ms], wd_sb[:, it, 0:512],
                            start=(it == 0), stop=(it == IT - 1),
                        )
                    y0 = spool.tile([P, 512], F32, tag="y0")
                    nc.scalar.copy(y0[:], py0[:])
                    nc.sync.dma_start(y_d[ms, 0:512], y0[:])
                    py1 = psum.tile([P, 512], F32, tag="py1")
                    for it in range(IT):
                        nc.tensor.matmul(
                            py1[:], hT_sb[:, it, ms], wd_sb[:, it, 512:1024],
                            start=(it == 0), stop=(it == IT - 1),
                        )
                    # the very last tile: two half copies on different
                    # engines + two DMA descriptors so the final transfer
                    # parallelizes across DMA engines
                    y1 = spool.tile([P, 512], F32, tag="y1")
                    nc.vector.tensor_copy(y1[:, 0:256], py1[:, 0:256])
                    nc.sync.dma_start(y_d[ms, 512:768], y1[:, 0:256])
                    nc.scalar.copy(y1[:, 256:512], py1[:, 256:512])
                    nc.sync.dma_start(y_d[ms, 768:1024], y1[:, 256:512])

    nc.compile()
    return nc


def _get_compiled():
    global _compiled
    if _compiled is None:
        _compiled = _build()
    return _compiled


def _numpy_fallback(hidden_states, gate_kernel, up_kernel, down_kernel, group_sizes):
    # Exact reference math on host; only used for unexpected group_sizes.
    out = np.empty((hidden_states.shape[0], down_kernel.shape[2]), np.float32)
    start = 0
    for e in range(gate_kernel.shape[0]):
        g = int(group_sizes[e])
        x = hidden_states[start:start + g]
        gate = x @ gate_kernel[e]
        up = x @ up_kernel[e]
        h = (gate / (1.0 + np.exp(-gate))) * up
        out[start:start + g] = h @ down_kernel[e]
        start += g
    out[start:] = 0.0
    return out


def _make_in_maps(hidden_states, gate_kernel, up_kernel, down_kernel):
    bf = ml_dtypes.bfloat16
    in_maps = []
    for e in range(E):
        x_e = hidden_states[e * T:(e + 1) * T]
        in_maps.append({
            "xT": np.ascontiguousarray(x_e.T).astype(bf),
            "wg": np.ascontiguousarray(gate_kernel[e]).astype(bf),
            "wu": np.ascontiguousarray(up_kernel[e]).astype(bf),
            "wd": np.ascontiguousarray(down_kernel[e]).astype(bf),
        })
    return in_maps


def profile_run(inputs, tmpdir=None):
    """Dev helper (not used by grading): run with NTFF tracing, return exec ns."""
    nc = _get_compiled()
    in_maps = _make_in_maps(
        np.asarray(inputs["hidden_states"], np.float32),
        np.asarray(inputs["gate_kernel"], np.float32),
        np.asarray(inputs["up_kernel"], np.float32),
        np.asarray(inputs["down_kernel"], np.float32),
    )
    res = run_bass_kernel_spmd(
        nc, in_maps, core_ids=list(range(E)), trace=True, tmpdir=tmpdir
    )
    return res.exec_time_ns


def kernel(hidden_states, gate_kernel, up_kernel, down_kernel, group_sizes):
    hidden_states = np.asarray(hidden_states, dtype=np.float32)
    gate_kernel = np.asarray(gate_kernel, dtype=np.float32)
    up_kernel = np.asarray(up_kernel, dtype=np.float32)
    down_kernel = np.asarray(down_kernel, dtype=np.float32)
    gs = np.asarray(group_sizes)

    if not (gs.shape == (E,) and np.all(gs == T)):
        return _numpy_fallback(
            hidden_states, gate_kernel, up_kernel, down_kernel, gs
        )

    nc = _get_compiled()
    in_maps = _make_in_maps(hidden_states, gate_kernel, up_kernel, down_kernel)
    res = run_bass_kernel_spmd(nc, in_maps, core_ids=list(range(E)))
    return np.concatenate([res.results[e]["y"] for e in range(E)], axis=0)


# revision 14
# speedup vs baseline: 1.1580x; 1.0082x over previous
"""Trainium2 Bass kernel for Llama4TextExperts-style grouped MoE FFN (SwiGLU).

Full-input contract: kernel(**inputs) takes the complete unsharded tensors and
returns the full [4096, 1024] output. Internally: expert-parallel across the 8
NeuronCores — core e gets expert e's three weight matrices and that expert's
512-token group (tokens arrive pre-sorted by expert with equal group sizes).
All routing / transposition is done host-side in numpy; no collectives needed.

Per-core device program (three GEMMs, ~6.4 GFLOP):
  phase 1: gate^T = Wg^T-stationary @ x^T, up^T likewise; SwiGLU fused on
           ACT (Silu) + DVE (mul) into h^T [I, T] bf16 resident in SBUF.
  phase 2: y = h @ Wd with h^T slices stationary, Wd streaming from its
           natural [I, H] DRAM layout; y lands untransposed in PSUM.
"""

import numpy as np
import ml_dtypes

import concourse.bass as bass
import concourse.mybir as mybir
import concourse.tile as tile
from concourse.tile import add_dep_helper
from concourse import bacc
from concourse.bass_utils import run_bass_kernel_spmd

# Problem shape (hardcoded per contract)
E = 8          # experts == cores
T = 512        # tokens per expert group
H = 1024       # hidden
I = 2048       # intermediate
P = 128        # partitions
KT = H // P    # 8  k-tiles over hidden
IT = I // P    # 16 i-tiles over intermediate
WB = 4         # i-blocks of 512 columns for gate/up weight streaming
MT = T // P    # 4  token tiles

BF16 = mybir.dt.bfloat16
F32 = mybir.dt.float32

_compiled = None  # (nc, ) cached across calls


def _build():
    nc = bacc.Bacc(None)
    xT_d = nc.declare_dram_parameter("xT", [H, T], BF16, isOutput=False)
    wg_d = nc.declare_dram_parameter("wg", [H, I], BF16, isOutput=False)
    wu_d = nc.declare_dram_parameter("wu", [H, I], BF16, isOutput=False)
    wd_d = nc.declare_dram_parameter("wd", [I, H], BF16, isOutput=False)
    y_d = nc.declare_dram_parameter("y", [T, H], F32, isOutput=True)

    xT_r = xT_d.rearrange("(ko p) t -> p ko t", p=P)     # [128, 8, 512]
    wg_r = wg_d.rearrange("(ko p) i -> p ko i", p=P)     # [128, 8, 2048]
    wu_r = wu_d.rearrange("(ko p) i -> p ko i", p=P)
    wd_r = wd_d.rearrange("(io p) h -> p io h", p=P)     # [128, 16, 1024]

    with tile.TileContext(nc) as tc:
        with (
            tc.tile_pool(name="xpool", bufs=1) as xpool,
            tc.tile_pool(name="wdpool", bufs=1) as wdpool,
            tc.tile_pool(name="hpool", bufs=1) as hpool,
            tc.tile_pool(name="wpool", bufs=3) as wpool,
            tc.tile_pool(name="spool", bufs=3) as spool,
            tc.tile_pool(name="psum", bufs=2, space="PSUM") as psum,
        ):
            # Startup-critical loads. The DMA engines round-robin across all
            # outstanding descriptors (everything in flight completes
            # together at ~320GB/s aggregate), so later weight loads are
            # GATED on earlier DMA completions: the pipe first carries only
            # the ~1.5MB the first i-tile needs, then stays about one
            # compute-block ahead.
            wg_it = []   # per-i-tile gate weights for block 0
            wu_it = []
            d_stage = []  # last DMA instruction of each stage, for gating
            wg_it.append(xpool.tile([P, KT, 128], BF16, tag="wg_it0", name="wg_it0"))
            d0g = nc.sync.dma_start(wg_it[0][:], wg_r[:, :, 0:128])
            wu_it.append(xpool.tile([P, KT, 128], BF16, tag="wu_it0", name="wu_it0"))
            d0u = nc.sync.dma_start(wu_it[0][:], wu_r[:, :, 0:128])

            xk4 = []
            for kq in range(4):
                xc = xpool.tile([P, 2, T], BF16, tag=f"x{kq}", name=f"x{kq}")
                # alternate x^T chunks between the scalar HWDGE queue and the
                # gpsimd SWDGE queue: more descriptors in flight early while
                # the issue rate (~0.65us per descriptor per queue) ramps
                eng = nc.scalar if kq % 2 == 0 else nc.gpsimd
                eng.dma_start(xc[:], xT_r[:, 2 * kq:2 * kq + 2, :])
                xk4.append(xc)

            def xk(kt):
                return xk4[kt // 2][:, kt % 2, :]

            hT_sb = hpool.tile([P, IT, T], BF16)
            wd_sb = wdpool.tile([P, IT, H], BF16)

            def gated(dma_call, stage_idx, dep=None):
                if dep is not None:
                    add_dep_helper(
                        dma_call.ins, dep.ins, reason="dma staging throttle"
                    )
                elif stage_idx >= 0:
                    add_dep_helper(
                        dma_call.ins, d_stage[stage_idx].ins,
                        reason="dma staging throttle",
                    )
                return dma_call

            # stage A (ungated, with xT): it1 of block 0
            wg_it.append(xpool.tile([P, KT, 128], BF16, tag="wg_it1", name="wg_it1"))
            gated(nc.sync.dma_start(wg_it[1][:], wg_r[:, :, 128:256]), -1)
            wu_it.append(xpool.tile([P, KT, 128], BF16, tag="wu_it1", name="wu_it1"))
            d_stage.append(
                gated(nc.sync.dma_start(wu_it[1][:], wu_r[:, :, 128:256]), -1)
            )
            # it2+it3 of block 0, gated on the first it0 load (lookahead
            # gating keeps the pipe from draining dry between stages)
            wg_b0r = xpool.tile([P, KT, 256], BF16, tag="wg_b0r")
            gated(nc.sync.dma_start(wg_b0r[:], wg_r[:, :, 256:512]), -2, d0g)
            wu_b0r = xpool.tile([P, KT, 256], BF16, tag="wu_b0r")
            d_stage.append(
                gated(nc.sync.dma_start(wu_b0r[:], wu_r[:, :, 256:512]), -2, d0g)
            )
            # blocks 1..3: block wb gated on the stage two back
            wgf, wuf = {}, {}
            for wb in range(1, WB):
                wgb = wpool.tile([P, KT, 512], BF16, tag="wgf")
                gated(
                    nc.sync.dma_start(
                        wgb[:], wg_r[:, :, wb * 512:(wb + 1) * 512]
                    ),
                    wb - 1,
                )
                wgf[wb] = wgb
                wub = wpool.tile([P, KT, 512], BF16, tag="wuf")
                d_stage.append(
                    gated(
                        nc.sync.dma_start(
                            wub[:], wu_r[:, :, wb * 512:(wb + 1) * 512]
                        ),
                        wb - 1,
                    )
                )
                wuf[wb] = wub
            # Wd halves, chained behind the weight stream
            d_stage.append(
                gated(
                    nc.sync.dma_start(wd_sb[:, 0:8, :], wd_r[:, 0:8, :]), 3
                )
            )
            gated(nc.sync.dma_start(wd_sb[:, 8:16, :], wd_r[:, 8:16, :]), 4)

            def gu_slice(wb, itl, which):
                if wb == 0:
                    if itl < 2:
                        t = wg_it[itl] if which == "g" else wu_it[itl]
                        return lambda kt: t[:, kt, :]
                    t = wg_b0r if which == "g" else wu_b0r
                    return lambda kt: t[:, kt, (itl - 2) * P:(itl - 1) * P]
                t = wgf[wb] if which == "g" else wuf[wb]
                return lambda kt: t[:, kt, itl * P:(itl + 1) * P]

            for wb in range(WB):
                for itl in range(4):
                    it = wb * 4 + itl
                    gsl = gu_slice(wb, itl, "g")
                    usl = gu_slice(wb, itl, "u")
                    pg = psum.tile([P, T], F32, tag="pg")
                    pu = psum.tile([P, T], F32, tag="pu")
                    for kt in range(KT):
                        nc.tensor.matmul(
                            pg[:], gsl(kt), xk(kt),
                            start=(kt == 0), stop=(kt == KT - 1),
                        )
                    for kt in range(KT):
                        nc.tensor.matmul(
                            pu[:], usl(kt), xk(kt),
                            start=(kt == 0), stop=(kt == KT - 1),
                        )
                    sg = spool.tile([P, T], F32)
                    nc.scalar.activation(
                        sg[:], pg[:], mybir.ActivationFunctionType.Silu
                    )
                    nc.vector.tensor_mul(hT_sb[:, it, :], sg[:], pu[:])

            for mt in range(MT):
                ms = slice(mt * P, (mt + 1) * P)
                if mt < MT - 1:
                    py0 = psum.tile([P, 512], F32, tag="py0")
                    py1 = psum.tile([P, 512], F32, tag="py1")
                    for it in range(IT):
                        lhsT = hT_sb[:, it, ms]
                        nc.tensor.matmul(
                            py0[:], lhsT, wd_sb[:, it, 0:512],
                            start=(it == 0), stop=(it == IT - 1),
                        )
                        nc.tensor.matmul(
                            py1[:], lhsT, wd_sb[:, it, 512:1024],
                            start=(it == 0), stop=(it == IT - 1),
                        )
                    y0 = spool.tile([P, 512], F32, tag="y0")
                    nc.scalar.copy(y0[:], py0[:])
                    nc.sync.dma_start(y_d[ms, 0:512], y0[:])
                    y1 = spool.tile([P, 512], F32, tag="y1")
                    nc.vector.tensor_copy(y1[:], py1[:])
                    nc.sync.dma_start(y_d[ms, 512:1024], y1[:])
                else:
                    # last token tile: run the two 16-matmul chains
                    # back-to-back instead of interleaved, so the first
                    # half's copy+DMA overlaps the second half's matmuls and
                    # only one [128,512] copy+DMA remains after the last MM.
                    py0 = psum.tile([P, 512], F32, tag="py0")
                    for it in range(IT):
                        nc.tensor.matmul(
                            py0[:], hT_sb[:, it, ms], wd_sb[:, it, 0:512],
                            start=(it == 0), stop=(it == IT - 1),
                        )
                    y0 = spool.tile([P, 512], F32, tag="y0")
                    nc.scalar.copy(y0[:], py0[:])
                    nc.sync.dma_start(y_d[ms, 0:512], y0[:])
                    py1 = psum.tile([P, 512], F32, tag="py1")
                    for it in range(IT):
                        nc.tensor.matmul(
                            py1[:], hT_sb[:, it, ms], wd_sb[:, it, 512:1024],
                            start=(it == 0), stop=(it == IT - 1),
                        )
                    # the very last tile: two half copies on different
                    # engines + two DMA descriptors so the final transfer
                    # parallelizes across DMA engines
                    y1 = spool.tile([P, 512], F32, tag="y1")
                    nc.vector.tensor_copy(y1[:, 0:256], py1[:, 0:256])
                    nc.sync.dma_start(y_d[ms, 512:768], y1[:, 0:256])
                    nc.scalar.copy(y1[:, 256:512], py1[:, 256:512])
                    nc.sync.dma_start(y_d[ms, 768:1024], y1[:, 256:512])

    nc.compile()
    return nc


def _get_compiled():
    global _compiled
    if _compiled is None:
        _compiled = _build()
    return _compiled


def _numpy_fallback(hidden_states, gate_kernel, up_kernel, down_kernel, group_sizes):
    # Exact reference math on host; only used for unexpected group_sizes.
    out = np.empty((hidden_states.shape[0], down_kernel.shape[2]), np.float32)
    start = 0
    for e in range(gate_kernel.shape[0]):
        g = int(group_sizes[e])
        x = hidden_states[start:start + g]
        gate = x @ gate_kernel[e]
        up = x @ up_kernel[e]
        h = (gate / (1.0 + np.exp(-gate))) * up
        out[start:start + g] = h @ down_kernel[e]
        start += g
    out[start:] = 0.0
    return out


def _make_in_maps(hidden_states, gate_kernel, up_kernel, down_kernel):
    bf = ml_dtypes.bfloat16
    in_maps = []
    for e in range(E):
        x_e = hidden_states[e * T:(e + 1) * T]
        in_maps.append({
            "xT": np.ascontiguousarray(x_e.T).astype(bf),
            "wg": np.ascontiguousarray(gate_kernel[e]).astype(bf),
            "wu": np.ascontiguousarray(up_kernel[e]).astype(bf),
            "wd": np.ascontiguousarray(down_kernel[e]).astype(bf),
        })
    return in_maps


def profile_run(inputs, tmpdir=None):
    """Dev helper (not used by grading): run with NTFF tracing, return exec ns."""
    nc = _get_compiled()
    in_maps = _make_in_maps(
        np.asarray(inputs["hidden_states"], np.float32),
        np.asarray(inputs["gate_kernel"], np.float32),
        np.asarray(inputs["up_kernel"], np.float32),
        np.asarray(inputs["down_kernel"], np.float32),
    )
    res = run_bass_kernel_spmd(
        nc, in_maps, core_ids=list(range(E)), trace=True, tmpdir=tmpdir
    )
    return res.exec_time_ns


def kernel(hidden_states, gate_kernel, up_kernel, down_kernel, group_sizes):
    hidden_states = np.asarray(hidden_states, dtype=np.float32)
    gate_kernel = np.asarray(gate_kernel, dtype=np.float32)
    up_kernel = np.asarray(up_kernel, dtype=np.float32)
    down_kernel = np.asarray(down_kernel, dtype=np.float32)
    gs = np.asarray(group_sizes)

    if not (gs.shape == (E,) and np.all(gs == T)):
        return _numpy_fallback(
            hidden_states, gate_kernel, up_kernel, down_kernel, gs
        )

    nc = _get_compiled()
    in_maps = _make_in_maps(hidden_states, gate_kernel, up_kernel, down_kernel)
    res = run_bass_kernel_spmd(nc, in_maps, core_ids=list(range(E)))
    return np.concatenate([res.results[e]["y"] for e in range(E)], axis=0)
